# revision 1
# baseline (speedup 1.0000x reference)
"""Permutation scatter: out[perm[i]] = inputs[i]  (B=131072, D=512, f32).

Since perm is a permutation, out[j] = inputs[inv_perm[j]] -- a pure row
gather.  Strategy: shard the OUTPUT rows across the 8 cores and replicate
the full input to every core.  Core k owns output rows [k*R, (k+1)*R) and
gathers its 16384 rows (2 KiB each) from its local replica with indirect
DMAs, then writes its output shard contiguously.  No collectives; per-core
HBM traffic is the minimum possible (32 MiB read + 32 MiB write).  The
host only computes the inverse permutation (index math); all payload
movement happens on-device.

HW contract for indirect DMA (probed): one index per partition, dest AP
[128, D].  So each gather chunk covers 128 output rows; 128 chunks/core.
The per-core index tensor is passed pre-transposed (idxT[p, c] =
inv_k[c*128 + p]) so a single contiguous [128, 128] SBUF tile holds one
chunk's indices per column.
"""

import numpy as np

B = 131072
D = 512
N_CORES = 8
R = B // N_CORES  # 16384 output rows per core
P = 128
NCH = R // P  # 128 chunks per core

DATA_BUFS = 12
USE_RAW = True
RAW_SLOTS = 24  # rotating [128, RAW_GROUP*D] tiles
RAW_GROUP = 1  # 128-row gathers per store (grouping measured no better)

_cached = None


def _build_nc(data_bufs=DATA_BUFS):
    import concourse.bacc as bacc
    import concourse.bass as bass
    import concourse.mybir as mybir
    import concourse.tile as tile

    nc = bacc.Bacc(
        "TRN2",
        target_bir_lowering=False,
        debug=False,
        num_devices=N_CORES,
    )

    x = nc.dram_tensor("x", [B, D], mybir.dt.float32, kind="ExternalInput")
    # idxT[p, c] = source row for output row c*128 + p (core-local)
    idxT = nc.dram_tensor("idxT", [P, NCH], mybir.dt.int32, kind="ExternalInput")
    y = nc.dram_tensor("y", [R, D], mybir.dt.float32, kind="ExternalOutput")

    y_r = y[:].rearrange("(c p) d -> c p d", p=P)

    with tile.TileContext(nc) as tc:
        with (
            tc.tile_pool(name="idxp", bufs=1) as ipool,
            tc.tile_pool(name="data", bufs=data_bufs) as dpool,
        ):
            it = ipool.tile([P, NCH], mybir.dt.int32)
            nc.sync.dma_start(out=it[:], in_=idxT[:])
            for c in range(NCH):
                dtile = dpool.tile([P, D], mybir.dt.float32)
                nc.gpsimd.indirect_dma_start(
                    out=dtile[:],
                    out_offset=None,
                    in_=x[:],
                    in_offset=bass.IndirectOffsetOnAxis(ap=it[:, c : c + 1], axis=0),
                )
                nc.sync.dma_start(out=y_r[c], in_=dtile[:])

    nc.compile()
    return nc


def _build_nc_raw(slots=RAW_SLOTS, group=RAW_GROUP):
    """Raw-Bass version (no TileContext): hand-rolled semaphores, minimal
    prologue/epilogue.  ``group`` 128-row gathers land in one [128, group*D]
    SBUF tile which is written back with a single large store (fewer SP
    instructions, bigger store descriptors).  ``slots`` tiles rotate."""
    from contextlib import ExitStack

    import concourse.bass as bass
    import concourse.mybir as mybir

    n_groups = NCH // group
    assert NCH % group == 0

    nc = bass.Bass(
        "TRN2",
        target_bir_lowering=False,
        debug=False,
        num_devices=N_CORES,
    )

    x = nc.dram_tensor("x", [B, D], mybir.dt.float32, kind="ExternalInput")
    idxT = nc.dram_tensor("idxT", [P, NCH], mybir.dt.int32, kind="ExternalInput")
    y = nc.dram_tensor("y", [R, D], mybir.dt.float32, kind="ExternalOutput")
    # Store target for group j: output rows [j*group*128, (j+1)*group*128),
    # with partition p holding the `group` CONSECUTIVE rows
    # [j*group*128 + p*group, j*group*128 + (p+1)*group) -- so each partition
    # writes one contiguous group*D*4-byte run (big store descriptors).
    # Gather g of the group fills tile columns [g*D, (g+1)*D), so its 128
    # indices must be inv_k[j*group*128 + p*group + g] (see _make_in_maps).
    y_r = y[:].rearrange("(j p g) d -> j p (g d)", p=P, g=group)

    with ExitStack() as ctx:
        it = ctx.enter_context(nc.sbuf_tensor("it", [P, NCH], mybir.dt.int32))
        dts = [
            ctx.enter_context(
                nc.sbuf_tensor(f"dt{i}", [P, group * D], mybir.dt.float32)
            )
            for i in range(slots)
        ]
        # Per-slot semaphores with exact thresholds (a single cumulative sem
        # is racy: completions from the 16 SDMA engines interleave across
        # successive DMAs).  A slot's store waits for all `group` gathers of
        # its round (full sum = race-free); the next round's gathers wait for
        # that store.
        isem = nc.alloc_semaphore("isem")
        isem2 = nc.alloc_semaphore("isem2")
        gsems = [nc.alloc_semaphore(f"gsem{i}") for i in range(slots)]
        ssems = [nc.alloc_semaphore(f"ssem{i}") for i in range(slots)]

        # Split the index load: a small head load unblocks the first gathers
        # ~1.5us earlier (the 64KB load's completion receipt gates gather 0).
        head_chunks = 8
        assert head_chunks % group == 0 and head_chunks < NCH

        def rounds(slot):  # number of groups handled by this slot
            return (n_groups - slot + slots - 1) // slots

        with nc.Block(no_gpsimd_drain=True) as block:

            @block.sync
            def _(sync):
                sync.dma_start(out=it[:, :head_chunks], in_=idxT[:, :head_chunks]).then_inc(isem, 16)
                sync.dma_start(out=it[:, head_chunks:], in_=idxT[:, head_chunks:]).then_inc(isem2, 16)
                for j in range(n_groups):
                    i, k = j % slots, j // slots
                    sync.wait_ge(gsems[i], (k + 1) * group * 16)
                    sync.dma_start(out=y_r[j], in_=dts[i][:]).then_inc(
                        ssems[i], 16
                    )
                for i in range(slots):
                    sync.wait_ge(ssems[i], rounds(i) * 16)
                sync.wait_ge(isem, 16)
                sync.wait_ge(isem2, 16)

            @block.gpsimd
            def _(g_):
                g_.wait_ge(isem, 16)
                for j in range(n_groups):
                    i, k = j % slots, j // slots
                    if j * group == head_chunks:
                        g_.wait_ge(isem2, 16)
                    if j >= slots:
                        g_.wait_ge(ssems[i], k * 16)
                    for g in range(group):
                        c = j * group + g
                        g_.indirect_dma_start(
                            out=dts[i][:, g * D : (g + 1) * D],
                            out_offset=None,
                            in_=x[:],
                            in_offset=bass.IndirectOffsetOnAxis(
                                ap=it[:, c : c + 1], axis=0
                            ),
                        ).then_inc(gsems[i], 16)

        # Block exit emitted per-engine drains + a sem-only barrier; all DMA
        # completions were explicitly waited above, so a plain range-clear
        # (no dge drain) suffices to make the NEFF re-executable.
        sem_nums = sorted(
            [isem.num, isem2.num]
            + [s.num for s in gsems]
            + [s.num for s in ssems]
        )
        assert sem_nums == list(range(sem_nums[0], sem_nums[-1] + 1))
        nc.gpsimd.sem_clear(range(sem_nums[0], sem_nums[-1] + 1))

    return nc


def _get_nc():
    global _cached
    if _cached is None:
        _cached = _build_nc_raw() if USE_RAW else _build_nc()
    return _cached


def _make_in_maps(inputs, perm):
    x = np.ascontiguousarray(np.asarray(inputs, dtype=np.float32))
    p = np.asarray(perm).astype(np.int64)
    inv = np.empty(B, dtype=np.int32)
    inv[p] = np.arange(B, dtype=np.int32)
    maps = []
    for k in range(N_CORES):
        sl = inv[k * R : (k + 1) * R]
        if USE_RAW:
            # idxT[p, j*group + g] = inv_k[j*group*128 + p*group + g]
            n_groups = NCH // RAW_GROUP
            idxT = (
                sl.reshape(n_groups, P, RAW_GROUP)
                .transpose(1, 0, 2)
                .reshape(P, NCH)
            )
        else:
            # idxT[p, c] = inv_k[c*128 + p]
            idxT = sl.reshape(NCH, P).T
        maps.append({"x": x, "idxT": np.ascontiguousarray(idxT)})
    return maps


def kernel(**kw):
    from concourse.bass_utils import run_bass_kernel_spmd

    nc = _get_nc()
    in_maps = _make_in_maps(kw["inputs"], kw["perm"])
    res = run_bass_kernel_spmd(nc, in_maps, core_ids=list(range(N_CORES)))
    return np.concatenate([res.results[k]["y"] for k in range(N_CORES)], axis=0)


def run_traced(inputs, perm, **trace_kw):
    """test.py helper: same as kernel() but returns (out, BassKernelResults)."""
    from concourse.bass_utils import run_bass_kernel_spmd

    nc = _get_nc()
    in_maps = _make_in_maps(inputs, perm)
    res = run_bass_kernel_spmd(
        nc, in_maps, core_ids=list(range(N_CORES)), trace=True, **trace_kw
    )
    out = np.concatenate([res.results[k]["y"] for k in range(N_CORES)], axis=0)
    return out, res

